# revision 33
# baseline (speedup 1.0000x reference)
"""Trainium2 Bass kernel for the siamese-kNN classification head.

Reference computation (B=256, N=2000, D=512, C=100):
    scores[b,n] = sigmoid(sum_d w_d * |a[b,d] - S[n,d]| + kb)
    out[b,c]    = (scores @ L)[b,c] / count_c     (0 where count_c == 0)

Strategy
--------
Data-parallel over the batch: core i handles rows 32*i..32*i+32, no
collectives.  |x| = relu(2x) - x splits the score into a nonlinear slab
(relu(A''-S''), A''=2|w|(.)a, S''=2|w|(.)S) plus a separable linear part
folded into a tiny rank-2 f32r correction matmul.  d lives on partitions
(4 chunks of 128), n on the free dim; PE reduces each slab over d into
PSUM via a sliding-window sign stationary that routes row b's reduction
to output row b.

New in this version (vs the 133us baseline):
  * fp8 DoubleRow hybrid: ~58% of the 128 per-core slabs are produced in
    fp8e4 and consumed in PAIRS by DoubleRow matmuls ([128,2,500] moving,
    [128,2,32] sign window stationary) -- one MM ingests TWO batch rows'
    slabs for one chunk (k0=row 2j, k1=row 2j+1), measured at the same
    211ns spacing as a bf16 MM = 2x PE throughput.  Remaining slabs stay
    bf16 (DVE 4x mode is 1.8x faster at producing bf16 than fp8, so an
    all-fp8 kernel would be production-bound).  Mix solves the 3-engine
    balance: PE .422*n8+.844*n16, DVE 1.171*d8+.65*d16, ACT 1.994*a8,
    optimum n8=74, n16=54, ACT 38 of the fp8 sub-slabs -> all ~77us.
  * e4m3 slab rounding adds ~1e-2 worst-case output rel err (vs 2e-2
    gate, host-simulated 1.48e-2 for all-fp8, less for the hybrid).
  * Head: constant DMAs issue in parallel from 5 engine queues (the
    ~655ns/issue serial chain on Sync was 4.7us of the old critical
    path); chunk 0 of S'' is split across two queues.
  * PE warm-up: 6 dummy matmuls on a memset scratch tile keep the PE HAM
    busy from the preamble end so real matmuls run at 2.4GHz, and an
    early dummy activation hides the 1.3us ACT table load.
"""

import sys

for _p in ("/opt/trn_rl_repo", "/root/.axon_site/_ro/trn_rl_repo"):
    if _p not in sys.path:
        sys.path.append(_p)

import numpy as np

B, N, D, C = 256, 2000, 512, 100
NP = 2048                  # label rows padded to 16 full chunks
NCORES = 8
BSH = B // NCORES          # 32 batch rows per core
DCH = D // 128             # 4 d-chunks
NSEG = 4                   # PSUM free-dim segments
SEG = N // NSEG            # 500
NLAB = NP // 128           # 16 label chunks
NDUMMY = 8                 # PE warm-up matmuls

# ---- hybrid schedule ------------------------------------------------------
# Per chunk: PAIRS[ch] fp8 b-pairs (pair j covers rows 2j, 2j+1), rest bf16.
# Sub-slab producers: 'A' = ACT (+relu, +sign), 'V' = DVE (-relu, -sign).
# Combos below index the fp8 sign tiles: 0=(V,V), 1=(A,A), 2=(A,V).
# Mix solves the 3-engine balance at the P0-throttled measured rates:
# PE .524*n8+1.032*n16, DVE 1.41*d8+0.80*d16, ACT 2.224*a8 -> n8=78, a8=41.
NPAIRS = (10, 10, 10, 9)          # fp8 pairs per chunk (39 total = 78 slabs)
NACT_PURE = (5, 6, 5, 4)          # leading pairs produced fully on ACT
NMIXED = (0, 0, 1, 0)             # next pair: ACT k0 + DVE k1
NTAIL = 3                         # chunk-3 trailing bf16 units (seg-major end)
COMBO_VV, COMBO_AA, COMBO_AV = 0, 1, 2

_CACHE = {}


def _interleave(b16, vpairs):
    out = []
    nv, nb, vi = len(vpairs), max(1, len(b16)), 0
    for i, u in enumerate(b16):
        out.append(u)
        while vi < ((i + 1) * nv) // nb:
            out.append(vpairs[vi])
            vi += 1
    out.extend(vpairs[vi:])
    return out


def _chunk_units(ch):
    """Emission order for one chunk: interleave so instantaneous engine
    rates stay balanced (ACT delivers a pair only every ~4.5us)."""
    np_, na, nm = NPAIRS[ch], NACT_PURE[ch], NMIXED[ch]
    apairs = [("p8", j, COMBO_AA) for j in range(na)]
    apairs += [("p8", na + i, COMBO_AV) for i in range(nm)]
    vpairs = [("p8", na + nm + i, COMBO_VV) for i in range(np_ - na - nm)]
    b16 = [("b16", b) for b in range(2 * np_, BSH)]
    if ch == 0:
        # ramp: bf16-only until production/ACT catch up, first ACT pair late
        backbone = b16[:5] + _interleave(b16[5:], vpairs)
        off = 1
    elif ch == DCH - 1:
        # ending: pairs early, NTAIL bf16 units last (seg-major epilogue)
        backbone = _interleave(b16[:-NTAIL], vpairs)
        off = 0
    else:
        backbone = _interleave(b16, vpairs)
        off = 0
    total = len(backbone) + len(apairs)
    out = list(backbone)
    for ai in range(len(apairs) - 1, -1, -1):
        pos = min(len(out), ((ai + 1 + off) * total) // (len(apairs) + 1 + off))
        out.insert(pos, apairs[ai])
    if ch == DCH - 1:
        out.extend(b16[-NTAIL:])
    return out


def _split_multi_waits(nc):
    """TRN2 TPB instructions encode at most ONE semaphore wait; split extras
    into single-wait NOPs directly before the instruction (same engine)."""
    from concourse import mybir

    for fn in nc.m.functions:
        for bb in fn.blocks:
            out = []
            for inst in bb.instructions:
                si = inst.sync_info
                if si is not None and si.on_wait and len(si.on_wait) > 1:
                    waits = list(si.on_wait)
                    for j, w in enumerate(waits[:-1]):
                        out.append(mybir.InstNoOp(
                            name=f"{inst.name}-sw{j}", engine=inst.engine,
                            sync_info=mybir.SyncInfo(on_wait=[w], on_update=[]),
                            ins=[], outs=[]))
                    inst.sync_info = mybir.SyncInfo(
                        on_wait=[waits[-1]], on_update=list(si.on_update))
                out.append(inst)
            bb.instructions = out


def _build_nc():
    import concourse.bass as bass
    import concourse.tile as tile
    from concourse import mybir

    f32 = mybir.dt.float32
    f32r = mybir.dt.float32r
    bf16 = mybir.dt.bfloat16
    fp8 = mybir.dt.float8e4
    DR = mybir.MatmulPerfMode.DoubleRow
    nc = bass.Bass()

    # s2t split into contiguous pieces so every DMA reads full-rate DRAM and
    # ring-FIFO order gives strict priority: chunk-0 halves first.
    s2t0a_d = nc.declare_dram_parameter("s2t0a", [128, 1000], bf16, isOutput=False)
    s2t0b_d = nc.declare_dram_parameter("s2t0b", [128, 1000], bf16, isOutput=False)
    s2t1_d = nc.declare_dram_parameter("s2t1", [128, N], bf16, isOutput=False)
    s2t23_d = nc.declare_dram_parameter("s2t23", [2, 128, N], bf16, isOutput=False)
    # a2t: [p, ch*BSH+b] = 2|w|(.)a ;  recb: reciprocal class counts [BSH, C]
    a2t_d = nc.declare_dram_parameter("a2t", [128, 128], f32, isOutput=False)
    recb_d = nc.declare_dram_parameter("recb", [BSH, C], f32, isOutput=False)
    # f32r pack: [2, N+BSH]; [:, :N] = (1, wS)^T rows, [:, N:] = (kb-wa, 1)
    cpack_d = nc.declare_dram_parameter("cpack", [2, N + BSH], f32r, isOutput=False)
    # bf16 pack: sgnn [128,DCH,63] | ident rows0:32 [32]
    bf16p_d = nc.declare_dram_parameter("bf16p", [128, DCH * 63 + 32], bf16,
                                        isOutput=False)
    # fp8 sign windows [128, 3 combos, DCH, 2, 64]
    fp8p_d = nc.declare_dram_parameter("fp8p", [128, 3 * DCH * 2 * 64], fp8,
                                       isOutput=False)
    lab_d = nc.declare_dram_parameter("labels", [128, NLAB * C], bf16, isOutput=False)
    out_d = nc.declare_dram_parameter("out", [BSH, C], f32, isOutput=True)

    with tile.TileContext(nc) as tc:
        with (
            tc.tile_pool(name="const", bufs=1) as const,
            tc.tile_pool(name="b16pool", bufs=9) as b16pool,
            tc.tile_pool(name="p8pool", bufs=7) as p8pool,
            tc.tile_pool(name="bank", bufs=8, space="PSUM") as bankp,
        ):
            # ---------------- tiles ----------------
            scratch = const.tile([128, 512], bf16, name="scratch", tag="scratch")
            actscr = const.tile([128, 8], bf16, name="actscr", tag="actscr")
            s2t = const.tile([128, DCH * N], bf16, name="s2t", tag="s2t")
            a2t = const.tile([128, 128], f32, name="a2t", tag="a2t")
            recb = const.tile([BSH, C], f32, name="recb", tag="recb")
            cpack = const.tile([2, N + BSH], f32r, name="cpack", tag="cpack")
            bf16p = const.tile([128, DCH * 63 + 32], bf16, name="bf16p",
                               tag="bf16p")
            fp8p = const.tile([128, 3 * DCH * 2 * 64], fp8, name="fp8p", tag="fp8p")
            labs = const.tile([128, NLAB * C], bf16, name="labs", tag="labs")

            # ---------------- warm-up + staged parallel DMA issue ----------
            # The DMA fabric drains all active rings ~fairly at ~260GB/s, so
            # priority comes from ring-FIFO order: the critical transfers
            # (chunk-0 halves, a2t, bf16 signs) are each ring's head; the
            # 1.9MB of later-needed bulk (s2t1/s2t23/labels) sits at the
            # TAIL of the gpsimd ring where it cannot steal early bandwidth.
            nc.gpsimd.memset(scratch[:], 0.0)

            # sync ring: chunk-0 a2t columns, then chunk0 lo half
            nc.sync.dma_start(a2t[:, 0:BSH], a2t_d[:, 0:BSH])
            nc.sync.dma_start(s2t[:, 0:1000], s2t0a_d[:])
            # scalar ring: chunk0 hi half -> rest of a2t -> (dummy act below)
            nc.scalar.dma_start(s2t[:, 1000:2000], s2t0b_d[:])
            nc.scalar.dma_start(a2t[:, BSH:128], a2t_d[:, BSH:128])
            # gpsimd ring: small early constants, then the bulk
            nc.gpsimd.dma_start(bf16p[:], bf16p_d[:])
            nc.gpsimd.dma_start(cpack[:], cpack_d[:])
            nc.gpsimd.dma_start(fp8p[:], fp8p_d[:])
            nc.gpsimd.dma_start(recb[:], recb_d[:])
            nc.gpsimd.dma_start(s2t[:, N : 2 * N], s2t1_d[:])
            nc.gpsimd.dma_start(
                s2t[:, 2 * N : 4 * N].rearrange("p (c n) -> p c n", c=2),
                s2t23_d[:].rearrange("c p n -> p c n"))
            nc.gpsimd.dma_start(labs[:], lab_d[:])

            pscr = bankp.tile([128, 512], f32, name="pscr", tag="bank")
            for i in range(NDUMMY):
                nc.tensor.matmul(
                    pscr[:], scratch[:, 0:128], scratch[:, 0:512],
                    start=True, stop=True, skip_group_check=True)
            # dummy activation pulls the 1.5us ACT table load off the
            # critical path
            nc.scalar.activation(
                actscr[:], scratch[:, 0:8],
                mybir.ActivationFunctionType.Relu, bias=0.0, scale=-1.0)

            # ---------------- views ----------------
            sgn8 = fp8p[:].rearrange("p (c h k x) -> p c h k x", c=3, h=DCH, k=2)
            ident = bf16p[0:32, DCH * 63 : DCH * 63 + 32]

            psc = [
                bankp.tile([BSH, SEG], f32, name=f"psc{s}", tag="bank")
                for s in range(NSEG)
            ]

            # ---------------- main stream ----------------
            # Ramp: the first NSPLIT bf16 units use SEPARATE lo/hi half
            # tiles so their seg-0/1 matmuls depend only on the chunk-0 lo
            # half (first DMA to land) -- all lo work is emitted before any
            # hi work so neither FIFO stalls on the s2t0b transfer.
            NSPLIT = 3
            ch0_units = _chunk_units(0)
            split_bs = [u[1] for u in ch0_units[:NSPLIT]]
            assert all(u[0] == "b16" for u in ch0_units[:NSPLIT])
            sl_lo, sl_hi = [], []
            for b in split_bs:
                t = const.tile([128, 1000], bf16, name=f"slo{b}", tag=f"slo{b}")
                nc.vector.tensor_scalar(
                    t[:], s2t[:, 0:1000],
                    a2t[:, b : b + 1], 0.0,
                    mybir.AluOpType.subtract, mybir.AluOpType.min,
                )
                sl_lo.append(t)
            for i, b in enumerate(split_bs):
                lhs = bf16p[:, 31 - b : 63 - b]
                for s in (0, 1):
                    nc.tensor.matmul(
                        psc[s][:], lhs, sl_lo[i][:, SEG * s : SEG * (s + 1)],
                        start=(i == 0), stop=False, skip_group_check=True,
                    )
            for b in split_bs:
                t = const.tile([128, 1000], bf16, name=f"shi{b}", tag=f"shi{b}")
                nc.vector.tensor_scalar(
                    t[:], s2t[:, 1000:2000],
                    a2t[:, b : b + 1], 0.0,
                    mybir.AluOpType.subtract, mybir.AluOpType.min,
                )
                sl_hi.append(t)
            for i in range(2):  # bridge the s2t0b DMA latency
                nc.tensor.matmul(
                    pscr[:], scratch[:, 0:128], scratch[:, 0:512],
                    start=True, stop=True, skip_group_check=True)
            for i, b in enumerate(split_bs):
                lhs = bf16p[:, 31 - b : 63 - b]
                for s in (2, 3):
                    nc.tensor.matmul(
                        psc[s][:],
                        lhs, sl_hi[i][:, SEG * (s - 2) : SEG * (s - 1)],
                        start=(i == 0), stop=False, skip_group_check=True,
                    )

            first = False
            tail_units = _chunk_units(DCH - 1)[-NTAIL:]
            for ch in range(DCH):
                units = _chunk_units(ch)
                if ch == 0:
                    units = units[NSPLIT:]
                if ch == DCH - 1:
                    units = units[:-NTAIL]
                for ui, u in enumerate(units):
                    stop = False
                    if u[0] == "b16":
                        b = u[1]
                        slab = b16pool.tile([128, N], bf16, name="slab16",
                                            tag="slab16")
                        scal = a2t[:, ch * BSH + b : ch * BSH + b + 1]
                        lhs = bf16p[:, ch * 63 + 31 - b : ch * 63 + 63 - b]
                        nc.vector.tensor_scalar(
                            slab[:], s2t[:, ch * N : (ch + 1) * N],
                            scal, 0.0,
                            mybir.AluOpType.subtract, mybir.AluOpType.min,
                        )
                        for s in range(NSEG):
                            nc.tensor.matmul(
                                psc[s][:], lhs,
                                slab[:, SEG * s : SEG * (s + 1)],
                                start=first, stop=stop,
                                skip_group_check=True,
                            )
                    else:
                        _, j, combo = u
                        slab8 = p8pool.tile([128, 2, N], fp8, name="slab8",
                                            tag="slab8")
                        for k in range(2):
                            b = 2 * j + k
                            col = ch * BSH + b
                            use_act = (combo == COMBO_AA) or (
                                combo == COMBO_AV and k == 0)
                            if use_act:
                                nc.scalar.activation(
                                    slab8[:, k, :], s2t[:, ch * N : (ch + 1) * N],
                                    mybir.ActivationFunctionType.Relu,
                                    bias=a2t[:, col : col + 1], scale=-1.0,
                                )
                            else:
                                nc.vector.tensor_scalar(
                                    slab8[:, k, :], s2t[:, ch * N : (ch + 1) * N],
                                    a2t[:, col : col + 1], 0.0,
                                    mybir.AluOpType.subtract, mybir.AluOpType.min,
                                )
                        lhs = sgn8[:, combo, ch, :, 31 - 2 * j : 63 - 2 * j]
                        for s in range(NSEG):
                            nc.tensor.matmul(
                                psc[s][:], lhs,
                                slab8[:, :, SEG * s : SEG * (s + 1)],
                                start=first, stop=stop, perf_mode=DR,
                                skip_group_check=True,
                            )
                    first = False
                if ch == 1:
                    # rank-2 linear correction, mid-stream (warm, off the
                    # head/tail critical paths)
                    for s in range(NSEG):
                        nc.tensor.matmul(
                            psc[s][:], cpack[:, N : N + BSH],
                            cpack[:, SEG * s : SEG * (s + 1)],
                            start=False, stop=False, skip_group_check=True,
                        )

            # ---------------- seg-major epilogue ----------------
            # The last NTAIL bf16 units emit their matmuls seg-by-seg so
            # psc[0] completes ~5 units early; each segment's sigmoid,
            # transposes, PSUM->SBUF copy and label matmuls then pipeline
            # under the remaining segments' matmuls.
            ch = DCH - 1
            tails = []
            for u in tail_units:
                b = u[1]
                slab = b16pool.tile([128, N], bf16, name="slab16", tag="slab16")
                nc.vector.tensor_scalar(
                    slab[:], s2t[:, ch * N : (ch + 1) * N],
                    a2t[:, ch * BSH + b : ch * BSH + b + 1], 0.0,
                    mybir.AluOpType.subtract, mybir.AluOpType.min,
                )
                tails.append((b, slab))

            ssig = const.tile([BSH, N], bf16, name="ssig", tag="ssig")
            tpall = bankp.tile([128, NLAB * BSH], bf16, name="tpall", tag="bank")
            sct = const.tile([128, NLAB * BSH], bf16, name="sct", tag="sct")
            out_ps = bankp.tile([BSH, C], f32, name="out_ps", tag="bank")
            ktab = ((0, 3), (3, 7), (7, 11), (11, NLAB))

            def emit_transposes(s):
                for k in range(*ktab[s]):
                    pk = min(128, N - 128 * k)
                    nc.tensor.transpose(
                        tpall[:pk, BSH * k : BSH * k + BSH],
                        ssig[:, 128 * k : 128 * k + pk], ident,
                    )

            for s in range(NSEG):
                for i, (b, slab) in enumerate(tails):
                    lhs = bf16p[:, ch * 63 + 31 - b : ch * 63 + 63 - b]
                    nc.tensor.matmul(
                        psc[s][:], lhs, slab[:, SEG * s : SEG * (s + 1)],
                        start=False, stop=(i == len(tails) - 1),
                        skip_group_check=True,
                    )
                nc.scalar.activation(
                    ssig[:, SEG * s : SEG * (s + 1)], psc[s][:],
                    mybir.ActivationFunctionType.Sigmoid,
                )
                if s >= 2:
                    emit_transposes(s - 2)
            emit_transposes(2)
            emit_transposes(3)
            for g in range(NSEG):
                k0, k1 = ktab[g]
                nc.vector.tensor_copy(
                    sct[:, BSH * k0 : BSH * k1], tpall[:, BSH * k0 : BSH * k1])
            for k in range(NLAB):
                pk = min(128, N - 128 * k)
                nc.tensor.matmul(
                    out_ps[:], sct[:pk, BSH * k : BSH * k + BSH],
                    labs[:pk, k * C : (k + 1) * C],
                    start=(k == 0), stop=(k == NLAB - 1),
                )

            # ---------------- divide by counts, write out ----------------
            out_s = const.tile([BSH, C], f32, name="out_s", tag="out_s")
            nc.vector.tensor_mul(out_s[:], out_ps[:], recb[:])
            nc.sync.dma_start(out_d[:], out_s[:])

    _split_multi_waits(nc)
    return nc


def _prep_host(inputs, support_tensors, support_labels, kernel_w, kernel_b):
    import ml_dtypes

    bf16 = ml_dtypes.bfloat16
    fp8 = ml_dtypes.float8_e4m3fn
    a = np.asarray(inputs, dtype=np.float32)
    S = np.asarray(support_tensors, dtype=np.float32)
    L = np.asarray(support_labels, dtype=np.float32)
    w = np.asarray(kernel_w, dtype=np.float32)
    kb = np.float32(np.asarray(kernel_b, dtype=np.float32))

    aw = 2.0 * np.abs(w)
    sgn = np.sign(w).astype(np.float32)
    s2t = np.ascontiguousarray((S * aw[None, :]).T).astype(bf16)   # [D, N]
    wS = (S @ w).astype(np.float32)                                # [N]
    wa = (a @ w).astype(np.float32)                                # [B]
    a2 = a * aw[None, :]                                           # [B, D]

    sgn_chunks = sgn.reshape(DCH, 128).T                           # [128, DCH]
    # bf16 windows: col 31 = -sign (all bf16 slabs come from DVE) | ident
    bf16p = np.zeros((128, DCH * 63 + 32), dtype=np.float32)
    for ch in range(DCH):
        bf16p[:, ch * 63 + 31] = -sgn_chunks[:, ch]
    bf16p[:32, DCH * 63 :] = np.eye(32, dtype=np.float32)

    # fp8 sign windows [128, combo, ch, k, 64]: k0 sign at col 31 -> output
    # row 2j; k1 sign at col 32 -> output row 2j+1.  Polarity per producer:
    # DVE slabs hold -relu -> -sign;  ACT slabs hold +relu -> +sign.
    fp8p = np.zeros((128, 3, DCH, 2, 64), dtype=np.float32)
    for ch in range(DCH):
        s_ = sgn_chunks[:, ch]
        fp8p[:, COMBO_VV, ch, 0, 31] = -s_
        fp8p[:, COMBO_VV, ch, 1, 32] = -s_
        fp8p[:, COMBO_AA, ch, 0, 31] = s_
        fp8p[:, COMBO_AA, ch, 1, 32] = s_
        fp8p[:, COMBO_AV, ch, 0, 31] = s_      # k0 from ACT
        fp8p[:, COMBO_AV, ch, 1, 32] = -s_     # k1 from DVE
    fp8p = fp8p.reshape(128, -1).astype(fp8)

    cpack = np.empty((2, N + BSH), dtype=np.float32)
    cpack[0, :N] = 1.0
    cpack[1, :N] = wS

    labp = np.zeros((NP, C), dtype=np.float32)
    labp[:N] = L
    labp = np.ascontiguousarray(
        labp.reshape(NLAB, 128, C).transpose(1, 0, 2)).reshape(128, -1).astype(bf16)

    counts = L.sum(axis=0)
    recip = np.where(counts != 0, 1.0 / np.maximum(counts, 1e-30), 0.0)

    recb = np.broadcast_to(recip.astype(np.float32), (BSH, C)).copy()
    shared = {
        "s2t0a": np.ascontiguousarray(s2t[0:128, 0:1000]),
        "s2t0b": np.ascontiguousarray(s2t[0:128, 1000:2000]),
        "s2t1": np.ascontiguousarray(s2t[128:256, :]),
        "s2t23": np.ascontiguousarray(s2t[256:512, :].reshape(2, 128, N)),
        "bf16p": bf16p.astype(bf16), "fp8p": fp8p,
        "labels": labp, "recb": recb,
    }
    in_maps = []
    for c in range(NCORES):
        rows = slice(BSH * c, BSH * (c + 1))
        a2t_c = np.ascontiguousarray(
            a2[rows].T.reshape(DCH, 128, BSH).transpose(1, 0, 2).reshape(
                128, DCH * BSH))
        cpack_c = cpack.copy()
        cpack_c[0, N:] = kb - wa[rows]
        cpack_c[1, N:] = 1.0
        in_maps.append(dict(shared, a2t=a2t_c, cpack=cpack_c))
    return in_maps


def kernel(**inputs) -> np.ndarray:
    from concourse.bass_utils import run_bass_kernel_spmd

    if "nc" not in _CACHE:
        _CACHE["nc"] = _build_nc()
    nc = _CACHE["nc"]

    in_maps = _prep_host(
        inputs["inputs"], inputs["support_tensors"], inputs["support_labels"],
        inputs["kernel_w"], inputs["kernel_b"],
    )
    res = run_bass_kernel_spmd(nc, in_maps, list(range(NCORES)))
    return np.concatenate([res.results[i]["out"] for i in range(NCORES)], axis=0)


# revision 39
# speedup vs baseline: 1.0085x; 1.0085x over previous
"""Trainium2 Bass kernel for the siamese-kNN classification head.

Reference computation (B=256, N=2000, D=512, C=100):
    scores[b,n] = sigmoid(sum_d w_d * |a[b,d] - S[n,d]| + kb)
    out[b,c]    = (scores @ L)[b,c] / count_c     (0 where count_c == 0)

Strategy
--------
Data-parallel over the batch: core i handles rows 32*i..32*i+32, no
collectives.  |x| = relu(2x) - x splits the score into a nonlinear slab
(relu(A''-S''), A''=2|w|(.)a, S''=2|w|(.)S) plus a separable linear part
folded into a tiny rank-2 f32r correction matmul.  d lives on partitions
(4 chunks of 128), n on the free dim; PE reduces each slab over d into
PSUM via a sliding-window sign stationary that routes row b's reduction
to output row b.

New in this version (vs the 133us baseline):
  * fp8 DoubleRow hybrid: ~58% of the 128 per-core slabs are produced in
    fp8e4 and consumed in PAIRS by DoubleRow matmuls ([128,2,500] moving,
    [128,2,32] sign window stationary) -- one MM ingests TWO batch rows'
    slabs for one chunk (k0=row 2j, k1=row 2j+1), measured at the same
    211ns spacing as a bf16 MM = 2x PE throughput.  Remaining slabs stay
    bf16 (DVE 4x mode is 1.8x faster at producing bf16 than fp8, so an
    all-fp8 kernel would be production-bound).  Mix solves the 3-engine
    balance: PE .422*n8+.844*n16, DVE 1.171*d8+.65*d16, ACT 1.994*a8,
    optimum n8=74, n16=54, ACT 38 of the fp8 sub-slabs -> all ~77us.
  * e4m3 slab rounding adds ~1e-2 worst-case output rel err (vs 2e-2
    gate, host-simulated 1.48e-2 for all-fp8, less for the hybrid).
  * Head: constant DMAs issue in parallel from 5 engine queues (the
    ~655ns/issue serial chain on Sync was 4.7us of the old critical
    path); chunk 0 of S'' is split across two queues.
  * PE warm-up: 6 dummy matmuls on a memset scratch tile keep the PE HAM
    busy from the preamble end so real matmuls run at 2.4GHz, and an
    early dummy activation hides the 1.3us ACT table load.
"""

import sys

for _p in ("/opt/trn_rl_repo", "/root/.axon_site/_ro/trn_rl_repo"):
    if _p not in sys.path:
        sys.path.append(_p)

import numpy as np

B, N, D, C = 256, 2000, 512, 100
NP = 2048                  # label rows padded to 16 full chunks
NCORES = 8
BSH = B // NCORES          # 32 batch rows per core
DCH = D // 128             # 4 d-chunks
NSEG = 4                   # PSUM free-dim segments
SEG = N // NSEG            # 500
NLAB = NP // 128           # 16 label chunks
NDUMMY = 12                # PE warm-up matmuls

# ---- hybrid schedule ------------------------------------------------------
# Per chunk: PAIRS[ch] fp8 b-pairs (pair j covers rows 2j, 2j+1), rest bf16.
# Sub-slab producers: 'A' = ACT (+relu, +sign), 'V' = DVE (-relu, -sign).
# Combos below index the fp8 sign tiles: 0=(V,V), 1=(A,A), 2=(A,V).
# Mix solves the 3-engine balance at the P0-throttled measured rates:
# PE .524*n8+1.032*n16, DVE 1.41*d8+0.80*d16, ACT 2.224*a8 -> n8=78, a8=41.
NPAIRS = (10, 10, 10, 9)          # fp8 pairs per chunk (39 total = 78 slabs)
NACT_PURE = (5, 5, 5, 4)          # leading pairs produced fully on ACT
NMIXED = (0, 0, 1, 0)             # next pair: ACT k0 + DVE k1
NTAIL = 2                         # chunk-3 trailing bf16 units (seg-major end)
COMBO_VV, COMBO_AA, COMBO_AV = 0, 1, 2

_CACHE = {}


def _interleave(b16, vpairs):
    out = []
    nv, nb, vi = len(vpairs), max(1, len(b16)), 0
    for i, u in enumerate(b16):
        out.append(u)
        while vi < ((i + 1) * nv) // nb:
            out.append(vpairs[vi])
            vi += 1
    out.extend(vpairs[vi:])
    return out


def _chunk_units(ch):
    """Emission order for one chunk: interleave so instantaneous engine
    rates stay balanced (ACT delivers a pair only every ~4.5us)."""
    np_, na, nm = NPAIRS[ch], NACT_PURE[ch], NMIXED[ch]
    apairs = [("p8", j, COMBO_AA) for j in range(na)]
    apairs += [("p8", na + i, COMBO_AV) for i in range(nm)]
    vpairs = [("p8", na + nm + i, COMBO_VV) for i in range(np_ - na - nm)]
    b16 = [("b16", b) for b in range(2 * np_, BSH)]
    if ch == 0:
        # ramp: bf16-only until production/ACT catch up, first ACT pair late
        backbone = b16[:5] + _interleave(b16[5:], vpairs)
        off = 1
    elif ch == DCH - 1:
        # ending: pairs early, NTAIL bf16 units last (seg-major epilogue)
        backbone = _interleave(b16[:-NTAIL], vpairs)
        off = 0
    else:
        backbone = _interleave(b16, vpairs)
        off = 0
    total = len(backbone) + len(apairs)
    out = list(backbone)
    for ai in range(len(apairs) - 1, -1, -1):
        pos = min(len(out), ((ai + 1 + off) * total) // (len(apairs) + 1 + off))
        out.insert(pos, apairs[ai])
    if ch == DCH - 1:
        out.extend(b16[-NTAIL:])
    return out


def _split_multi_waits(nc):
    """TRN2 TPB instructions encode at most ONE semaphore wait; split extras
    into single-wait NOPs directly before the instruction (same engine)."""
    from concourse import mybir

    for fn in nc.m.functions:
        for bb in fn.blocks:
            out = []
            for inst in bb.instructions:
                si = inst.sync_info
                if si is not None and si.on_wait and len(si.on_wait) > 1:
                    waits = list(si.on_wait)
                    for j, w in enumerate(waits[:-1]):
                        out.append(mybir.InstNoOp(
                            name=f"{inst.name}-sw{j}", engine=inst.engine,
                            sync_info=mybir.SyncInfo(on_wait=[w], on_update=[]),
                            ins=[], outs=[]))
                    inst.sync_info = mybir.SyncInfo(
                        on_wait=[waits[-1]], on_update=list(si.on_update))
                out.append(inst)
            bb.instructions = out


def _build_nc():
    import concourse.bass as bass
    import concourse.tile as tile
    from concourse import mybir

    f32 = mybir.dt.float32
    f32r = mybir.dt.float32r
    bf16 = mybir.dt.bfloat16
    fp8 = mybir.dt.float8e4
    DR = mybir.MatmulPerfMode.DoubleRow
    nc = bass.Bass()

    # s2t split into contiguous pieces so every DMA reads full-rate DRAM and
    # ring-FIFO order gives strict priority: chunk-0 halves first.
    s2t0a_d = nc.declare_dram_parameter("s2t0a", [128, 1000], bf16, isOutput=False)
    s2t0b_d = nc.declare_dram_parameter("s2t0b", [128, 1000], bf16, isOutput=False)
    s2t1_d = nc.declare_dram_parameter("s2t1", [128, N], bf16, isOutput=False)
    s2t23_d = nc.declare_dram_parameter("s2t23", [2, 128, N], bf16, isOutput=False)
    # a2t: [p, ch*BSH+b] = 2|w|(.)a ;  recb: reciprocal class counts [BSH, C]
    a2t_d = nc.declare_dram_parameter("a2t", [128, 128], f32, isOutput=False)
    recb_d = nc.declare_dram_parameter("recb", [BSH, C], f32, isOutput=False)
    # f32r pack: [2, N+BSH]; [:, :N] = (1, wS)^T rows, [:, N:] = (kb-wa, 1)
    cpack_d = nc.declare_dram_parameter("cpack", [2, N + BSH], f32r, isOutput=False)
    # bf16 pack: sgnn [128,DCH,63] | ident rows0:32 [32]
    bf16p_d = nc.declare_dram_parameter("bf16p", [128, DCH * 63 + 32], bf16,
                                        isOutput=False)
    # fp8 sign windows [128, 3 combos, DCH, 2, 64]
    fp8p_d = nc.declare_dram_parameter("fp8p", [128, 3 * DCH * 2 * 64], fp8,
                                       isOutput=False)
    lab_d = nc.declare_dram_parameter("labels", [128, NLAB * C], bf16, isOutput=False)
    out_d = nc.declare_dram_parameter("out", [BSH, C], f32, isOutput=True)

    with tile.TileContext(nc) as tc:
        with (
            tc.tile_pool(name="const", bufs=1) as const,
            tc.tile_pool(name="b16pool", bufs=9) as b16pool,
            tc.tile_pool(name="p8pool", bufs=7) as p8pool,
            tc.tile_pool(name="bank", bufs=8, space="PSUM") as bankp,
        ):
            # ---------------- tiles ----------------
            scratch = const.tile([128, 512], bf16, name="scratch", tag="scratch")
            actscr = const.tile([128, 8], bf16, name="actscr", tag="actscr")
            s2t = const.tile([128, DCH * N], bf16, name="s2t", tag="s2t")
            a2t = const.tile([128, 128], f32, name="a2t", tag="a2t")
            recb = const.tile([BSH, C], f32, name="recb", tag="recb")
            cpack = const.tile([2, N + BSH], f32r, name="cpack", tag="cpack")
            bf16p = const.tile([128, DCH * 63 + 32], bf16, name="bf16p",
                               tag="bf16p")
            fp8p = const.tile([128, 3 * DCH * 2 * 64], fp8, name="fp8p", tag="fp8p")
            labs = const.tile([128, NLAB * C], bf16, name="labs", tag="labs")

            # ---------------- warm-up + staged parallel DMA issue ----------
            # The DMA fabric drains all active rings ~fairly at ~260GB/s, so
            # priority comes from ring-FIFO order: the critical transfers
            # (chunk-0 halves, a2t, bf16 signs) are each ring's head; the
            # 1.9MB of later-needed bulk (s2t1/s2t23/labels) sits at the
            # TAIL of the gpsimd ring where it cannot steal early bandwidth.
            nc.gpsimd.memset(scratch[:], 0.0)

            # sync ring: chunk-0 a2t columns, then chunk0 lo half
            nc.sync.dma_start(a2t[:, 0:BSH], a2t_d[:, 0:BSH])
            nc.sync.dma_start(s2t[:, 0:1000], s2t0a_d[:])
            # scalar ring: chunk0 hi half -> rest of a2t -> (dummy act below)
            nc.scalar.dma_start(s2t[:, 1000:2000], s2t0b_d[:])
            nc.scalar.dma_start(a2t[:, BSH:128], a2t_d[:, BSH:128])
            # gpsimd ring: small early constants, then the bulk
            nc.gpsimd.dma_start(bf16p[:], bf16p_d[:])
            nc.gpsimd.dma_start(cpack[:], cpack_d[:])
            nc.gpsimd.dma_start(fp8p[:], fp8p_d[:])
            nc.gpsimd.dma_start(recb[:], recb_d[:])
            nc.gpsimd.dma_start(s2t[:, N : 2 * N], s2t1_d[:])
            nc.gpsimd.dma_start(
                s2t[:, 2 * N : 4 * N].rearrange("p (c n) -> p c n", c=2),
                s2t23_d[:].rearrange("c p n -> p c n"))
            nc.gpsimd.dma_start(labs[:], lab_d[:])

            pscr = bankp.tile([128, 512], f32, name="pscr", tag="bank")
            for i in range(NDUMMY):
                nc.tensor.matmul(
                    pscr[:], scratch[:, 0:128], scratch[:, 0:512],
                    start=True, stop=True, skip_group_check=True)
            # dummy activation pulls the 1.5us ACT table load off the
            # critical path
            nc.scalar.activation(
                actscr[:], scratch[:, 0:8],
                mybir.ActivationFunctionType.Relu, bias=0.0, scale=-1.0)

            # ---------------- views ----------------
            sgn8 = fp8p[:].rearrange("p (c h k x) -> p c h k x", c=3, h=DCH, k=2)
            ident = bf16p[0:32, DCH * 63 : DCH * 63 + 32]

            psc = [
                bankp.tile([BSH, SEG], f32, name=f"psc{s}", tag="bank")
                for s in range(NSEG)
            ]

            # ---------------- main stream ----------------
            # Ramp: the first NSPLIT bf16 units produce their slabs as
            # separate lo/hi half tiles -- the lo halves depend only on the
            # first-landing chunk-0-lo DMA, keeping DVE busy through the
            # ~2.5us the chunk-0-hi transfer still needs.  The ACT-produced
            # first pair is halved the same way.
            NSPLIT = 5
            ch0_units = _chunk_units(0)
            split_bs = [u[1] for u in ch0_units[:NSPLIT]]
            assert all(u[0] == "b16" for u in ch0_units[:NSPLIT])
            sl_lo, sl_hi = [], []
            for b in split_bs:
                t = const.tile([128, 1000], bf16, name=f"slo{b}", tag=f"slo{b}")
                nc.vector.tensor_scalar(
                    t[:], s2t[:, 0:1000],
                    a2t[:, b : b + 1], 0.0,
                    mybir.AluOpType.subtract, mybir.AluOpType.min,
                )
                sl_lo.append(t)
            for b in split_bs:
                t = const.tile([128, 1000], bf16, name=f"shi{b}", tag=f"shi{b}")
                nc.vector.tensor_scalar(
                    t[:], s2t[:, 1000:2000],
                    a2t[:, b : b + 1], 0.0,
                    mybir.AluOpType.subtract, mybir.AluOpType.min,
                )
                sl_hi.append(t)
            for i, b in enumerate(split_bs):
                lhs = bf16p[:, 31 - b : 63 - b]
                for s in range(NSEG):
                    src = (sl_lo[i][:, SEG * s : SEG * (s + 1)] if s < 2
                           else sl_hi[i][:, SEG * (s - 2) : SEG * (s - 1)])
                    nc.tensor.matmul(
                        psc[s][:], lhs, src,
                        start=(i == 0), stop=False, skip_group_check=True,
                    )

            first = False
            first_aa_done = False
            tail_units = _chunk_units(DCH - 1)[-NTAIL:]
            for ch in range(DCH):
                units = _chunk_units(ch)
                if ch == 0:
                    units = units[NSPLIT:]
                if ch == DCH - 1:
                    units = units[:-NTAIL]
                for ui, u in enumerate(units):
                    stop = False
                    if u[0] == "b16":
                        b = u[1]
                        slab = b16pool.tile([128, N], bf16, name="slab16",
                                            tag="slab16")
                        scal = a2t[:, ch * BSH + b : ch * BSH + b + 1]
                        lhs = bf16p[:, ch * 63 + 31 - b : ch * 63 + 63 - b]
                        nc.vector.tensor_scalar(
                            slab[:], s2t[:, ch * N : (ch + 1) * N],
                            scal, 0.0,
                            mybir.AluOpType.subtract, mybir.AluOpType.min,
                        )
                        for s in range(NSEG):
                            nc.tensor.matmul(
                                psc[s][:], lhs,
                                slab[:, SEG * s : SEG * (s + 1)],
                                start=first, stop=stop,
                                skip_group_check=True,
                            )
                    else:
                        _, j, combo = u
                        slab8 = p8pool.tile([128, 2, N], fp8, name="slab8",
                                            tag="slab8")
                        if ch == 0 and combo == COMBO_AA and not first_aa_done:
                            # halve the very first ACT pair: the lo halves
                            # only need the first-landing chunk-0-lo DMA
                            first_aa_done = True
                            for h in (0, 1):
                                for k in range(2):
                                    col = 2 * j + k
                                    nc.scalar.activation(
                                        slab8[:, k, 1000 * h : 1000 * (h + 1)],
                                        s2t[:, 1000 * h : 1000 * (h + 1)],
                                        mybir.ActivationFunctionType.Relu,
                                        bias=a2t[:, col : col + 1], scale=-1.0,
                                    )
                            for s in range(NSEG):
                                nc.tensor.matmul(
                                    psc[s][:],
                                    sgn8[:, combo, ch, :, 31 - 2 * j : 63 - 2 * j],
                                    slab8[:, :, SEG * s : SEG * (s + 1)],
                                    start=first, stop=stop, perf_mode=DR,
                                    skip_group_check=True,
                                )
                            continue
                        for k in range(2):
                            b = 2 * j + k
                            col = ch * BSH + b
                            use_act = (combo == COMBO_AA) or (
                                combo == COMBO_AV and k == 0)
                            if use_act:
                                nc.scalar.activation(
                                    slab8[:, k, :], s2t[:, ch * N : (ch + 1) * N],
                                    mybir.ActivationFunctionType.Relu,
                                    bias=a2t[:, col : col + 1], scale=-1.0,
                                )
                            else:
                                nc.vector.tensor_scalar(
                                    slab8[:, k, :], s2t[:, ch * N : (ch + 1) * N],
                                    a2t[:, col : col + 1], 0.0,
                                    mybir.AluOpType.subtract, mybir.AluOpType.min,
                                )
                        lhs = sgn8[:, combo, ch, :, 31 - 2 * j : 63 - 2 * j]
                        for s in range(NSEG):
                            nc.tensor.matmul(
                                psc[s][:], lhs,
                                slab8[:, :, SEG * s : SEG * (s + 1)],
                                start=first, stop=stop, perf_mode=DR,
                                skip_group_check=True,
                            )
                    first = False
                if ch == 1:
                    # rank-2 linear correction, mid-stream (warm, off the
                    # head/tail critical paths)
                    for s in range(NSEG):
                        nc.tensor.matmul(
                            psc[s][:], cpack[:, N : N + BSH],
                            cpack[:, SEG * s : SEG * (s + 1)],
                            start=False, stop=False, skip_group_check=True,
                        )

            # ---------------- seg-major epilogue ----------------
            # The last NTAIL bf16 units emit their matmuls seg-by-seg so
            # psc[0] completes ~5 units early; each segment's sigmoid,
            # transposes, PSUM->SBUF copy and label matmuls then pipeline
            # under the remaining segments' matmuls.
            ch = DCH - 1
            tails = []
            for u in tail_units:
                b = u[1]
                slab = b16pool.tile([128, N], bf16, name="slab16", tag="slab16")
                nc.vector.tensor_scalar(
                    slab[:], s2t[:, ch * N : (ch + 1) * N],
                    a2t[:, ch * BSH + b : ch * BSH + b + 1], 0.0,
                    mybir.AluOpType.subtract, mybir.AluOpType.min,
                )
                tails.append((b, slab))

            ssig = const.tile([BSH, N], bf16, name="ssig", tag="ssig")
            tpall = bankp.tile([128, NLAB * BSH], bf16, name="tpall", tag="bank")
            sct = const.tile([128, NLAB * BSH], bf16, name="sct", tag="sct")
            out_ps = bankp.tile([BSH, C], f32, name="out_ps", tag="bank")
            ktab = ((0, 3), (3, 7), (7, 11), (11, NLAB))

            def emit_transposes(s):
                for k in range(*ktab[s]):
                    pk = min(128, N - 128 * k)
                    nc.tensor.transpose(
                        tpall[:pk, BSH * k : BSH * k + BSH],
                        ssig[:, 128 * k : 128 * k + pk], ident,
                    )

            for s in range(NSEG):
                for i, (b, slab) in enumerate(tails):
                    lhs = bf16p[:, ch * 63 + 31 - b : ch * 63 + 63 - b]
                    nc.tensor.matmul(
                        psc[s][:], lhs, slab[:, SEG * s : SEG * (s + 1)],
                        start=False, stop=(i == len(tails) - 1),
                        skip_group_check=True,
                    )
                nc.scalar.activation(
                    ssig[:, SEG * s : SEG * (s + 1)], psc[s][:],
                    mybir.ActivationFunctionType.Sigmoid,
                )
                if s >= 2:
                    emit_transposes(s - 2)
            emit_transposes(2)
            emit_transposes(3)
            for g in range(NSEG):
                k0, k1 = ktab[g]
                nc.vector.tensor_copy(
                    sct[:, BSH * k0 : BSH * k1], tpall[:, BSH * k0 : BSH * k1])
            for k in range(NLAB):
                pk = min(128, N - 128 * k)
                nc.tensor.matmul(
                    out_ps[:], sct[:pk, BSH * k : BSH * k + BSH],
                    labs[:pk, k * C : (k + 1) * C],
                    start=(k == 0), stop=(k == NLAB - 1),
                )

            # ---------------- divide by counts, write out ----------------
            out_s = const.tile([BSH, C], f32, name="out_s", tag="out_s")
            nc.vector.tensor_mul(out_s[:], out_ps[:], recb[:])
            nc.sync.dma_start(out_d[:], out_s[:])

    _split_multi_waits(nc)
    return nc


def _prep_host(inputs, support_tensors, support_labels, kernel_w, kernel_b):
    import ml_dtypes

    bf16 = ml_dtypes.bfloat16
    fp8 = ml_dtypes.float8_e4m3fn
    a = np.asarray(inputs, dtype=np.float32)
    S = np.asarray(support_tensors, dtype=np.float32)
    L = np.asarray(support_labels, dtype=np.float32)
    w = np.asarray(kernel_w, dtype=np.float32)
    kb = np.float32(np.asarray(kernel_b, dtype=np.float32))

    aw = 2.0 * np.abs(w)
    sgn = np.sign(w).astype(np.float32)
    s2t = np.ascontiguousarray((S * aw[None, :]).T).astype(bf16)   # [D, N]
    wS = (S @ w).astype(np.float32)                                # [N]
    wa = (a @ w).astype(np.float32)                                # [B]
    a2 = a * aw[None, :]                                           # [B, D]

    sgn_chunks = sgn.reshape(DCH, 128).T                           # [128, DCH]
    # bf16 windows: col 31 = -sign (all bf16 slabs come from DVE) | ident
    bf16p = np.zeros((128, DCH * 63 + 32), dtype=np.float32)
    for ch in range(DCH):
        bf16p[:, ch * 63 + 31] = -sgn_chunks[:, ch]
    bf16p[:32, DCH * 63 :] = np.eye(32, dtype=np.float32)

    # fp8 sign windows [128, combo, ch, k, 64]: k0 sign at col 31 -> output
    # row 2j; k1 sign at col 32 -> output row 2j+1.  Polarity per producer:
    # DVE slabs hold -relu -> -sign;  ACT slabs hold +relu -> +sign.
    fp8p = np.zeros((128, 3, DCH, 2, 64), dtype=np.float32)
    for ch in range(DCH):
        s_ = sgn_chunks[:, ch]
        fp8p[:, COMBO_VV, ch, 0, 31] = -s_
        fp8p[:, COMBO_VV, ch, 1, 32] = -s_
        fp8p[:, COMBO_AA, ch, 0, 31] = s_
        fp8p[:, COMBO_AA, ch, 1, 32] = s_
        fp8p[:, COMBO_AV, ch, 0, 31] = s_      # k0 from ACT
        fp8p[:, COMBO_AV, ch, 1, 32] = -s_     # k1 from DVE
    fp8p = fp8p.reshape(128, -1).astype(fp8)

    cpack = np.empty((2, N + BSH), dtype=np.float32)
    cpack[0, :N] = 1.0
    cpack[1, :N] = wS

    labp = np.zeros((NP, C), dtype=np.float32)
    labp[:N] = L
    labp = np.ascontiguousarray(
        labp.reshape(NLAB, 128, C).transpose(1, 0, 2)).reshape(128, -1).astype(bf16)

    counts = L.sum(axis=0)
    recip = np.where(counts != 0, 1.0 / np.maximum(counts, 1e-30), 0.0)

    recb = np.broadcast_to(recip.astype(np.float32), (BSH, C)).copy()
    shared = {
        "s2t0a": np.ascontiguousarray(s2t[0:128, 0:1000]),
        "s2t0b": np.ascontiguousarray(s2t[0:128, 1000:2000]),
        "s2t1": np.ascontiguousarray(s2t[128:256, :]),
        "s2t23": np.ascontiguousarray(s2t[256:512, :].reshape(2, 128, N)),
        "bf16p": bf16p.astype(bf16), "fp8p": fp8p,
        "labels": labp, "recb": recb,
    }
    in_maps = []
    for c in range(NCORES):
        rows = slice(BSH * c, BSH * (c + 1))
        a2t_c = np.ascontiguousarray(
            a2[rows].T.reshape(DCH, 128, BSH).transpose(1, 0, 2).reshape(
                128, DCH * BSH))
        cpack_c = cpack.copy()
        cpack_c[0, N:] = kb - wa[rows]
        cpack_c[1, N:] = 1.0
        in_maps.append(dict(shared, a2t=a2t_c, cpack=cpack_c))
    return in_maps


def kernel(**inputs) -> np.ndarray:
    from concourse.bass_utils import run_bass_kernel_spmd

    if "nc" not in _CACHE:
        _CACHE["nc"] = _build_nc()
    nc = _CACHE["nc"]

    in_maps = _prep_host(
        inputs["inputs"], inputs["support_tensors"], inputs["support_labels"],
        inputs["kernel_w"], inputs["kernel_b"],
    )
    res = run_bass_kernel_spmd(nc, in_maps, list(range(NCORES)))
    return np.concatenate([res.results[i]["out"] for i in range(NCORES)], axis=0)


# revision 42
# speedup vs baseline: 1.0180x; 1.0094x over previous
"""Trainium2 Bass kernel for the siamese-kNN classification head.

Reference computation (B=256, N=2000, D=512, C=100):
    scores[b,n] = sigmoid(sum_d w_d * |a[b,d] - S[n,d]| + kb)
    out[b,c]    = (scores @ L)[b,c] / count_c     (0 where count_c == 0)

Strategy
--------
Data-parallel over the batch: core i handles rows 32*i..32*i+32, no
collectives.  |x| = relu(2x) - x splits the score into a nonlinear slab
(relu(A''-S''), A''=2|w|(.)a, S''=2|w|(.)S) plus a separable linear part
folded into a tiny rank-2 f32r correction matmul.  d lives on partitions
(4 chunks of 128), n on the free dim; PE reduces each slab over d into
PSUM via a sliding-window sign stationary that routes row b's reduction
to output row b.

New in this version (vs the 133us baseline; measured 103.2us, 1.29x):
  * fp8 DoubleRow hybrid: 78 of the 128 per-core slabs are produced in
    fp8e4 and consumed in PAIRS by DoubleRow matmuls ([128,2,500] moving,
    [128,2,32] sign window stationary) -- one MM ingests TWO batch rows'
    slabs for one chunk (k0=row 2j, k1=row 2j+1) at the same per-MM
    spacing as a bf16 MM = 2x PE throughput.  The other 50 slabs stay
    bf16 (DVE 4x mode produces bf16 ~1.8x faster than fp8, so an
    all-fp8 kernel would be production-bound).  The 39 ACT-produced fp8
    sub-slabs use +relu/+sign, DVE ones -relu/-sign; the mix balances
    PE/DVE/ACT at the concurrency-throttled (~2.0GHz P0) measured rates.
  * e4m3 slab rounding: measured 1.53e-2 worst-case output rel err vs
    the 2e-2 gate (host-simulated 1.48e-2 for all-fp8).
  * Head: the DMA fabric drains rings ~fairly at ~260GB/s with a ~2.7us
    per-DMA floor, so constants load as contiguous per-piece params with
    ring-FIFO priority (chunk-0 halves + a2t first; 1.9MB of bulk at the
    gpsimd ring tail); the first 5 slabs are built as lo/hi half tiles
    so DVE starts on the first 256KB that lands.
  * PE warm-up: 12 dummy matmuls on a memset scratch tile keep the PE
    HAM activity window busy from the preamble end so real matmuls run
    at full clock, and an early dummy activation hides the 1.5us ACT
    table load.
  * Tail: the last 2 bf16 units emit matmuls seg-major, so each psc
    bank's sigmoid -> transposes -> PSUM copy -> label matmuls pipeline
    under the remaining segments' matmuls.
"""

import sys

for _p in ("/opt/trn_rl_repo", "/root/.axon_site/_ro/trn_rl_repo"):
    if _p not in sys.path:
        sys.path.append(_p)

import numpy as np

B, N, D, C = 256, 2000, 512, 100
NP = 2048                  # label rows padded to 16 full chunks
NCORES = 8
BSH = B // NCORES          # 32 batch rows per core
DCH = D // 128             # 4 d-chunks
NSEG = 4                   # PSUM free-dim segments
SEG = N // NSEG            # 500
NLAB = NP // 128           # 16 label chunks
NDUMMY = 12                # PE warm-up matmuls

# ---- hybrid schedule ------------------------------------------------------
# Per chunk: PAIRS[ch] fp8 b-pairs (pair j covers rows 2j, 2j+1), rest bf16.
# Sub-slab producers: 'A' = ACT (+relu, +sign), 'V' = DVE (-relu, -sign).
# Combos below index the fp8 sign tiles: 0=(V,V), 1=(A,A), 2=(A,V).
# Mix solves the 3-engine balance at the P0-throttled measured rates:
# PE .524*n8+1.032*n16, DVE 1.41*d8+0.80*d16, ACT 2.224*a8 -> n8=78, a8=41.
NPAIRS = (10, 10, 10, 9)          # fp8 pairs per chunk (39 total = 78 slabs)
NACT_PURE = (5, 5, 5, 4)          # leading pairs produced fully on ACT
NMIXED = (0, 0, 1, 1)             # next pair: ACT k0 + DVE k1
NTAIL = 2                         # chunk-3 trailing bf16 units (seg-major end)
COMBO_VV, COMBO_AA, COMBO_AV = 0, 1, 2

_CACHE = {}


def _interleave(b16, vpairs):
    out = []
    nv, nb, vi = len(vpairs), max(1, len(b16)), 0
    for i, u in enumerate(b16):
        out.append(u)
        while vi < ((i + 1) * nv) // nb:
            out.append(vpairs[vi])
            vi += 1
    out.extend(vpairs[vi:])
    return out


def _chunk_units(ch):
    """Emission order for one chunk: interleave so instantaneous engine
    rates stay balanced (ACT delivers a pair only every ~4.5us)."""
    np_, na, nm = NPAIRS[ch], NACT_PURE[ch], NMIXED[ch]
    apairs = [("p8", j, COMBO_AA) for j in range(na)]
    apairs += [("p8", na + i, COMBO_AV) for i in range(nm)]
    vpairs = [("p8", na + nm + i, COMBO_VV) for i in range(np_ - na - nm)]
    b16 = [("b16", b) for b in range(2 * np_, BSH)]
    if ch == 0:
        # ramp: bf16-only until production/ACT catch up, first ACT pair late
        backbone = b16[:5] + _interleave(b16[5:], vpairs)
        off = 1
    elif ch == DCH - 1:
        # ending: pairs early, NTAIL bf16 units last (seg-major epilogue)
        backbone = _interleave(b16[:-NTAIL], vpairs)
        off = 0
    else:
        backbone = _interleave(b16, vpairs)
        off = 0
    total = len(backbone) + len(apairs)
    out = list(backbone)
    for ai in range(len(apairs) - 1, -1, -1):
        pos = min(len(out), ((ai + 1 + off) * total) // (len(apairs) + 1 + off))
        out.insert(pos, apairs[ai])
    if ch == DCH - 1:
        out.extend(b16[-NTAIL:])
    return out


def _split_multi_waits(nc):
    """TRN2 TPB instructions encode at most ONE semaphore wait; split extras
    into single-wait NOPs directly before the instruction (same engine)."""
    from concourse import mybir

    for fn in nc.m.functions:
        for bb in fn.blocks:
            out = []
            for inst in bb.instructions:
                si = inst.sync_info
                if si is not None and si.on_wait and len(si.on_wait) > 1:
                    waits = list(si.on_wait)
                    for j, w in enumerate(waits[:-1]):
                        out.append(mybir.InstNoOp(
                            name=f"{inst.name}-sw{j}", engine=inst.engine,
                            sync_info=mybir.SyncInfo(on_wait=[w], on_update=[]),
                            ins=[], outs=[]))
                    inst.sync_info = mybir.SyncInfo(
                        on_wait=[waits[-1]], on_update=list(si.on_update))
                out.append(inst)
            bb.instructions = out


def _build_nc():
    import concourse.bass as bass
    import concourse.tile as tile
    from concourse import mybir

    f32 = mybir.dt.float32
    f32r = mybir.dt.float32r
    bf16 = mybir.dt.bfloat16
    fp8 = mybir.dt.float8e4
    DR = mybir.MatmulPerfMode.DoubleRow
    nc = bass.Bass()

    # s2t split into contiguous pieces so every DMA reads full-rate DRAM and
    # ring-FIFO order gives strict priority: chunk-0 halves first.
    s2t0a_d = nc.declare_dram_parameter("s2t0a", [128, 1000], bf16, isOutput=False)
    s2t0b_d = nc.declare_dram_parameter("s2t0b", [128, 1000], bf16, isOutput=False)
    s2t1_d = nc.declare_dram_parameter("s2t1", [128, N], bf16, isOutput=False)
    s2t23_d = nc.declare_dram_parameter("s2t23", [2, 128, N], bf16, isOutput=False)
    # a2t: [p, ch*BSH+b] = 2|w|(.)a ;  recb: reciprocal class counts [BSH, C]
    a2t_d = nc.declare_dram_parameter("a2t", [128, 128], f32, isOutput=False)
    recb_d = nc.declare_dram_parameter("recb", [BSH, C], f32, isOutput=False)
    # f32r pack: [2, N+BSH]; [:, :N] = (1, wS)^T rows, [:, N:] = (kb-wa, 1)
    cpack_d = nc.declare_dram_parameter("cpack", [2, N + BSH], f32r, isOutput=False)
    # bf16 pack: sgnn [128,DCH,63] | ident rows0:32 [32]
    bf16p_d = nc.declare_dram_parameter("bf16p", [128, DCH * 63 + 32], bf16,
                                        isOutput=False)
    # fp8 sign windows [128, 3 combos, DCH, 2, 64]
    fp8p_d = nc.declare_dram_parameter("fp8p", [128, 3 * DCH * 2 * 64], fp8,
                                       isOutput=False)
    lab_d = nc.declare_dram_parameter("labels", [128, NLAB * C], bf16, isOutput=False)
    out_d = nc.declare_dram_parameter("out", [BSH, C], f32, isOutput=True)

    with tile.TileContext(nc) as tc:
        with (
            tc.tile_pool(name="const", bufs=1) as const,
            tc.tile_pool(name="b16pool", bufs=9) as b16pool,
            tc.tile_pool(name="p8pool", bufs=7) as p8pool,
            tc.tile_pool(name="bank", bufs=8, space="PSUM") as bankp,
        ):
            # ---------------- tiles ----------------
            scratch = const.tile([128, 512], bf16, name="scratch", tag="scratch")
            actscr = const.tile([128, 8], bf16, name="actscr", tag="actscr")
            s2t = const.tile([128, DCH * N], bf16, name="s2t", tag="s2t")
            a2t = const.tile([128, 128], f32, name="a2t", tag="a2t")
            recb = const.tile([BSH, C], f32, name="recb", tag="recb")
            cpack = const.tile([2, N + BSH], f32r, name="cpack", tag="cpack")
            bf16p = const.tile([128, DCH * 63 + 32], bf16, name="bf16p",
                               tag="bf16p")
            fp8p = const.tile([128, 3 * DCH * 2 * 64], fp8, name="fp8p", tag="fp8p")
            labs = const.tile([128, NLAB * C], bf16, name="labs", tag="labs")

            # ---------------- warm-up + staged parallel DMA issue ----------
            # The DMA fabric drains all active rings ~fairly at ~260GB/s, so
            # priority comes from ring-FIFO order: the critical transfers
            # (chunk-0 halves, a2t, bf16 signs) are each ring's head; the
            # 1.9MB of later-needed bulk (s2t1/s2t23/labels) sits at the
            # TAIL of the gpsimd ring where it cannot steal early bandwidth.
            nc.gpsimd.memset(scratch[:], 0.0)

            # sync ring: chunk-0 a2t columns, then chunk0 lo half
            nc.sync.dma_start(a2t[:, 0:BSH], a2t_d[:, 0:BSH])
            nc.sync.dma_start(s2t[:, 0:1000], s2t0a_d[:])
            # scalar ring: chunk0 hi half -> rest of a2t -> (dummy act below)
            nc.scalar.dma_start(s2t[:, 1000:2000], s2t0b_d[:])
            nc.scalar.dma_start(a2t[:, BSH:128], a2t_d[:, BSH:128])
            # gpsimd ring: small early constants, then the bulk
            nc.gpsimd.dma_start(bf16p[:], bf16p_d[:])
            nc.gpsimd.dma_start(cpack[:], cpack_d[:])
            nc.gpsimd.dma_start(fp8p[:], fp8p_d[:])
            nc.gpsimd.dma_start(recb[:], recb_d[:])
            nc.gpsimd.dma_start(s2t[:, N : 2 * N], s2t1_d[:])
            nc.gpsimd.dma_start(
                s2t[:, 2 * N : 4 * N].rearrange("p (c n) -> p c n", c=2),
                s2t23_d[:].rearrange("c p n -> p c n"))
            nc.gpsimd.dma_start(labs[:], lab_d[:])

            pscr = bankp.tile([128, 512], f32, name="pscr", tag="bank")
            for i in range(NDUMMY):
                nc.tensor.matmul(
                    pscr[:], scratch[:, 0:128], scratch[:, 0:512],
                    start=True, stop=True, skip_group_check=True)
            # dummy activation pulls the 1.5us ACT table load off the
            # critical path
            nc.scalar.activation(
                actscr[:], scratch[:, 0:8],
                mybir.ActivationFunctionType.Relu, bias=0.0, scale=-1.0)

            # ---------------- views ----------------
            sgn8 = fp8p[:].rearrange("p (c h k x) -> p c h k x", c=3, h=DCH, k=2)
            ident = bf16p[0:32, DCH * 63 : DCH * 63 + 32]

            psc = [
                bankp.tile([BSH, SEG], f32, name=f"psc{s}", tag="bank")
                for s in range(NSEG)
            ]

            # ---------------- main stream ----------------
            # Ramp: the first NSPLIT bf16 units produce their slabs as
            # separate lo/hi half tiles -- the lo halves depend only on the
            # first-landing chunk-0-lo DMA, keeping DVE busy through the
            # ~2.5us the chunk-0-hi transfer still needs.  The ACT-produced
            # first pair is halved the same way.
            NSPLIT = 5
            ch0_units = _chunk_units(0)
            split_bs = [u[1] for u in ch0_units[:NSPLIT]]
            assert all(u[0] == "b16" for u in ch0_units[:NSPLIT])
            sl_lo, sl_hi = [], []
            for b in split_bs:
                t = const.tile([128, 1000], bf16, name=f"slo{b}", tag=f"slo{b}")
                nc.vector.tensor_scalar(
                    t[:], s2t[:, 0:1000],
                    a2t[:, b : b + 1], 0.0,
                    mybir.AluOpType.subtract, mybir.AluOpType.min,
                )
                sl_lo.append(t)
            for b in split_bs:
                t = const.tile([128, 1000], bf16, name=f"shi{b}", tag=f"shi{b}")
                nc.vector.tensor_scalar(
                    t[:], s2t[:, 1000:2000],
                    a2t[:, b : b + 1], 0.0,
                    mybir.AluOpType.subtract, mybir.AluOpType.min,
                )
                sl_hi.append(t)
            for i, b in enumerate(split_bs):
                lhs = bf16p[:, 31 - b : 63 - b]
                for s in range(NSEG):
                    src = (sl_lo[i][:, SEG * s : SEG * (s + 1)] if s < 2
                           else sl_hi[i][:, SEG * (s - 2) : SEG * (s - 1)])
                    nc.tensor.matmul(
                        psc[s][:], lhs, src,
                        start=(i == 0), stop=False, skip_group_check=True,
                    )

            first = False
            first_aa_done = False
            tail_units = _chunk_units(DCH - 1)[-NTAIL:]
            for ch in range(DCH):
                units = _chunk_units(ch)
                if ch == 0:
                    units = units[NSPLIT:]
                if ch == DCH - 1:
                    units = units[:-NTAIL]
                for ui, u in enumerate(units):
                    stop = False
                    if u[0] == "b16":
                        b = u[1]
                        slab = b16pool.tile([128, N], bf16, name="slab16",
                                            tag="slab16")
                        scal = a2t[:, ch * BSH + b : ch * BSH + b + 1]
                        lhs = bf16p[:, ch * 63 + 31 - b : ch * 63 + 63 - b]
                        nc.vector.tensor_scalar(
                            slab[:], s2t[:, ch * N : (ch + 1) * N],
                            scal, 0.0,
                            mybir.AluOpType.subtract, mybir.AluOpType.min,
                        )
                        for s in range(NSEG):
                            nc.tensor.matmul(
                                psc[s][:], lhs,
                                slab[:, SEG * s : SEG * (s + 1)],
                                start=first, stop=stop,
                                skip_group_check=True,
                            )
                    else:
                        _, j, combo = u
                        slab8 = p8pool.tile([128, 2, N], fp8, name="slab8",
                                            tag="slab8")
                        if ch == 0 and combo == COMBO_AA and not first_aa_done:
                            # halve the very first ACT pair: the lo halves
                            # only need the first-landing chunk-0-lo DMA
                            first_aa_done = True
                            for h in (0, 1):
                                for k in range(2):
                                    col = 2 * j + k
                                    nc.scalar.activation(
                                        slab8[:, k, 1000 * h : 1000 * (h + 1)],
                                        s2t[:, 1000 * h : 1000 * (h + 1)],
                                        mybir.ActivationFunctionType.Relu,
                                        bias=a2t[:, col : col + 1], scale=-1.0,
                                    )
                            for s in range(NSEG):
                                nc.tensor.matmul(
                                    psc[s][:],
                                    sgn8[:, combo, ch, :, 31 - 2 * j : 63 - 2 * j],
                                    slab8[:, :, SEG * s : SEG * (s + 1)],
                                    start=first, stop=stop, perf_mode=DR,
                                    skip_group_check=True,
                                )
                            continue
                        for k in range(2):
                            b = 2 * j + k
                            col = ch * BSH + b
                            use_act = (combo == COMBO_AA) or (
                                combo == COMBO_AV and k == 0)
                            if use_act:
                                nc.scalar.activation(
                                    slab8[:, k, :], s2t[:, ch * N : (ch + 1) * N],
                                    mybir.ActivationFunctionType.Relu,
                                    bias=a2t[:, col : col + 1], scale=-1.0,
                                )
                            else:
                                nc.vector.tensor_scalar(
                                    slab8[:, k, :], s2t[:, ch * N : (ch + 1) * N],
                                    a2t[:, col : col + 1], 0.0,
                                    mybir.AluOpType.subtract, mybir.AluOpType.min,
                                )
                        lhs = sgn8[:, combo, ch, :, 31 - 2 * j : 63 - 2 * j]
                        for s in range(NSEG):
                            nc.tensor.matmul(
                                psc[s][:], lhs,
                                slab8[:, :, SEG * s : SEG * (s + 1)],
                                start=first, stop=stop, perf_mode=DR,
                                skip_group_check=True,
                            )
                    first = False
                if ch == 1:
                    # rank-2 linear correction, mid-stream (warm, off the
                    # head/tail critical paths)
                    for s in range(NSEG):
                        nc.tensor.matmul(
                            psc[s][:], cpack[:, N : N + BSH],
                            cpack[:, SEG * s : SEG * (s + 1)],
                            start=False, stop=False, skip_group_check=True,
                        )

            # ---------------- seg-major epilogue ----------------
            # The last NTAIL bf16 units emit their matmuls seg-by-seg so
            # psc[0] completes ~5 units early; each segment's sigmoid,
            # transposes, PSUM->SBUF copy and label matmuls then pipeline
            # under the remaining segments' matmuls.
            ch = DCH - 1
            tails = []
            for u in tail_units:
                b = u[1]
                slab = b16pool.tile([128, N], bf16, name="slab16", tag="slab16")
                nc.vector.tensor_scalar(
                    slab[:], s2t[:, ch * N : (ch + 1) * N],
                    a2t[:, ch * BSH + b : ch * BSH + b + 1], 0.0,
                    mybir.AluOpType.subtract, mybir.AluOpType.min,
                )
                tails.append((b, slab))

            ssig = const.tile([BSH, N], bf16, name="ssig", tag="ssig")
            tpall = bankp.tile([128, NLAB * BSH], bf16, name="tpall", tag="bank")
            sct = const.tile([128, NLAB * BSH], bf16, name="sct", tag="sct")
            out_ps = bankp.tile([BSH, C], f32, name="out_ps", tag="bank")
            ktab = ((0, 3), (3, 7), (7, 11), (11, NLAB))

            def emit_transposes(s):
                for k in range(*ktab[s]):
                    pk = min(128, N - 128 * k)
                    nc.tensor.transpose(
                        tpall[:pk, BSH * k : BSH * k + BSH],
                        ssig[:, 128 * k : 128 * k + pk], ident,
                    )

            for s in range(NSEG):
                for i, (b, slab) in enumerate(tails):
                    lhs = bf16p[:, ch * 63 + 31 - b : ch * 63 + 63 - b]
                    nc.tensor.matmul(
                        psc[s][:], lhs, slab[:, SEG * s : SEG * (s + 1)],
                        start=False, stop=(i == len(tails) - 1),
                        skip_group_check=True,
                    )
                nc.scalar.activation(
                    ssig[:, SEG * s : SEG * (s + 1)], psc[s][:],
                    mybir.ActivationFunctionType.Sigmoid,
                )
                if s >= 2:
                    emit_transposes(s - 2)
            emit_transposes(2)
            emit_transposes(3)
            for g in range(NSEG):
                k0, k1 = ktab[g]
                nc.vector.tensor_copy(
                    sct[:, BSH * k0 : BSH * k1], tpall[:, BSH * k0 : BSH * k1])
            for k in range(NLAB):
                pk = min(128, N - 128 * k)
                nc.tensor.matmul(
                    out_ps[:], sct[:pk, BSH * k : BSH * k + BSH],
                    labs[:pk, k * C : (k + 1) * C],
                    start=(k == 0), stop=(k == NLAB - 1),
                )

            # ---------------- divide by counts, write out ----------------
            out_s = const.tile([BSH, C], f32, name="out_s", tag="out_s")
            nc.vector.tensor_mul(out_s[:], out_ps[:], recb[:])
            nc.sync.dma_start(out_d[:], out_s[:], single_packet=True)

    _split_multi_waits(nc)
    return nc


def _prep_host(inputs, support_tensors, support_labels, kernel_w, kernel_b):
    import ml_dtypes

    bf16 = ml_dtypes.bfloat16
    fp8 = ml_dtypes.float8_e4m3fn
    a = np.asarray(inputs, dtype=np.float32)
    S = np.asarray(support_tensors, dtype=np.float32)
    L = np.asarray(support_labels, dtype=np.float32)
    w = np.asarray(kernel_w, dtype=np.float32)
    kb = np.float32(np.asarray(kernel_b, dtype=np.float32))

    aw = 2.0 * np.abs(w)
    sgn = np.sign(w).astype(np.float32)
    s2t = np.ascontiguousarray((S * aw[None, :]).T).astype(bf16)   # [D, N]
    wS = (S @ w).astype(np.float32)                                # [N]
    wa = (a @ w).astype(np.float32)                                # [B]
    a2 = a * aw[None, :]                                           # [B, D]

    sgn_chunks = sgn.reshape(DCH, 128).T                           # [128, DCH]
    # bf16 windows: col 31 = -sign (all bf16 slabs come from DVE) | ident
    bf16p = np.zeros((128, DCH * 63 + 32), dtype=np.float32)
    for ch in range(DCH):
        bf16p[:, ch * 63 + 31] = -sgn_chunks[:, ch]
    bf16p[:32, DCH * 63 :] = np.eye(32, dtype=np.float32)

    # fp8 sign windows [128, combo, ch, k, 64]: k0 sign at col 31 -> output
    # row 2j; k1 sign at col 32 -> output row 2j+1.  Polarity per producer:
    # DVE slabs hold -relu -> -sign;  ACT slabs hold +relu -> +sign.
    fp8p = np.zeros((128, 3, DCH, 2, 64), dtype=np.float32)
    for ch in range(DCH):
        s_ = sgn_chunks[:, ch]
        fp8p[:, COMBO_VV, ch, 0, 31] = -s_
        fp8p[:, COMBO_VV, ch, 1, 32] = -s_
        fp8p[:, COMBO_AA, ch, 0, 31] = s_
        fp8p[:, COMBO_AA, ch, 1, 32] = s_
        fp8p[:, COMBO_AV, ch, 0, 31] = s_      # k0 from ACT
        fp8p[:, COMBO_AV, ch, 1, 32] = -s_     # k1 from DVE
    fp8p = fp8p.reshape(128, -1).astype(fp8)

    cpack = np.empty((2, N + BSH), dtype=np.float32)
    cpack[0, :N] = 1.0
    cpack[1, :N] = wS

    labp = np.zeros((NP, C), dtype=np.float32)
    labp[:N] = L
    labp = np.ascontiguousarray(
        labp.reshape(NLAB, 128, C).transpose(1, 0, 2)).reshape(128, -1).astype(bf16)

    counts = L.sum(axis=0)
    recip = np.where(counts != 0, 1.0 / np.maximum(counts, 1e-30), 0.0)

    recb = np.broadcast_to(recip.astype(np.float32), (BSH, C)).copy()
    shared = {
        "s2t0a": np.ascontiguousarray(s2t[0:128, 0:1000]),
        "s2t0b": np.ascontiguousarray(s2t[0:128, 1000:2000]),
        "s2t1": np.ascontiguousarray(s2t[128:256, :]),
        "s2t23": np.ascontiguousarray(s2t[256:512, :].reshape(2, 128, N)),
        "bf16p": bf16p.astype(bf16), "fp8p": fp8p,
        "labels": labp, "recb": recb,
    }
    in_maps = []
    for c in range(NCORES):
        rows = slice(BSH * c, BSH * (c + 1))
        a2t_c = np.ascontiguousarray(
            a2[rows].T.reshape(DCH, 128, BSH).transpose(1, 0, 2).reshape(
                128, DCH * BSH))
        cpack_c = cpack.copy()
        cpack_c[0, N:] = kb - wa[rows]
        cpack_c[1, N:] = 1.0
        in_maps.append(dict(shared, a2t=a2t_c, cpack=cpack_c))
    return in_maps


def kernel(**inputs) -> np.ndarray:
    from concourse.bass_utils import run_bass_kernel_spmd

    if "nc" not in _CACHE:
        _CACHE["nc"] = _build_nc()
    nc = _CACHE["nc"]

    in_maps = _prep_host(
        inputs["inputs"], inputs["support_tensors"], inputs["support_labels"],
        inputs["kernel_w"], inputs["kernel_b"],
    )
    res = run_bass_kernel_spmd(nc, in_maps, list(range(NCORES)))
    return np.concatenate([res.results[i]["out"] for i in range(NCORES)], axis=0)
